# revision 7
# baseline (speedup 1.0000x reference)
"""Trainium2 Bass kernel for a transformer block: DyT-prenorm attention (RoPE,
causal+mask) + top-2-of-16 MoE with a shared expert.

Strategy (8 NeuronCores, SPMD single program, per-core data):
  * Attention head-parallel: core c computes head c (QK packed in one lhsT,
    rope via two elementwise passes), produces avT [64, S] and AllGathers it
    (bf16) so every core can run the wo projection + residual + dyt locally.
  * Router replicated (fp32 for top-2 selection stability); gate_w columns are
    permuted per core so the local experts are always columns 0 and 1.
  * Experts expert-parallel: 2 dense experts per core (gate weights folded in
    via a rank-1 broadcast matmul), shared expert sharded over FF (64 cols per
    core); everything accumulates into 4 PSUM tiles which are ReduceScattered
    (fp32) across cores; core 0 also contributes x + attention (residual).
  * All big matmuls bf16 with fp32 PSUM accumulation.
Everything is computed transposed (d on partitions, tokens on the free axis);
the host transposes the output back.
"""

import os
import numpy as np
import ml_dtypes

BF = ml_dtypes.bfloat16

S = 512      # tokens (B=1)
Dm = 512     # d_model
H = 8        # heads
HD = 64      # head dim
E = 16       # experts
FF = 512     # expert hidden
P = 128
NCORES = 8
DC = Dm // P    # 4 d-model chunks
TCH = S // P    # 4 token chunks
FCH = FF // P   # 4 ff chunks
SHF = FF // NCORES  # shared-expert ff slice per core (64)
DSH = Dm // NCORES  # output row shard per core (64)

GELU_C = float(2.0 * np.sqrt(2.0 / np.pi))  # sigmoid-form tanh-gelu scale
GELU_A = 0.044715

_PROG_CACHE = {}

LAST_INFO = {}


def _build_program(a1v: float, a2v: float, full_mask: bool):
    import concourse.bass as bass
    import concourse.mybir as mybir
    import concourse.tile as tile
    from concourse import bacc

    f32 = mybir.dt.float32
    bf16 = mybir.dt.bfloat16
    Alu = mybir.AluOpType
    Act = mybir.ActivationFunctionType
    AX = mybir.AxisListType
    ts = bass.ts

    nc = bacc.Bacc(
        "TRN2", target_bir_lowering=False, debug=False, num_devices=NCORES
    )

    def inp(name, shape, dt=f32):
        return nc.dram_tensor(name, list(shape), dt, kind="ExternalInput").ap()

    xT32_d = inp("xT32", (DC, P, S))
    wqk16_d = inp("wqk16", (DC, P, P), bf16)          # [d-chunk][d128, q64|k64]
    wv16_d = inp("wv16", (DC, P, HD), bf16)
    wo16_d = inp("wo16", (DC, P, Dm), bf16)           # wo row-chunks
    cd16_d = inp("cd16", (P, S), bf16)                # rope cos (stacked x4)
    cs16_d = inp("cs16", (P, S), bf16)                # rope +-sin (stacked)
    mcols = S if full_mask else P
    mask16_d = inp("mask16", (TCH, P, mcols), bf16)
    g1_d = inp("g1c", (DC, P, 1))
    b1_d = inp("b1c", (DC, P, 1))
    g2_d = inp("g2c", (DC, P, 1))
    b2_d = inp("b2c", (DC, P, 1))
    gw32_d = inp("gw32", (DC, P, E))                  # per-core permuted cols
    gb_d = inp("gb", (P, TCH, E))
    ones16_d = inp("ones16", (1, P), bf16)
    sel_d = inp("sel", (P, 1))                        # 1.0 on core 0 else 0.0
    id128_d = inp("id128", (P, P))
    ek16_d = inp("ek16", (2, DC, P, FF), bf16)        # local experts, up
    ev16_d = inp("ev16", (2, FCH, P, Dm), bf16)       # local experts, down
    sk16_d = inp("sk16", (DC, P, SHF), bf16)          # shared up slice
    sv16_d = inp("sv16", (SHF, Dm), bf16)             # shared down slice

    outT_d = nc.dram_tensor("outT", [DSH, S], f32, kind="ExternalOutput").ap()

    with tile.TileContext(nc, num_cores=NCORES) as tc:
        with (
            tc.tile_pool(name="cst", bufs=1) as cst,
            tc.tile_pool(name="tmp", bufs=3) as tmp,
            tc.tile_pool(name="ps", bufs=2, space="PSUM") as psp,
            tc.tile_pool(name="dram", bufs=1, space="DRAM") as drp,
        ):
            # ---------- constants / weights to SBUF ----------
            def load(dram_ap, shape, dt, tg):
                t = cst.tile(shape, dt, name=tg, tag=tg)
                nc.sync.dma_start(t[:], dram_ap)
                return t

            cd16 = load(cd16_d, (P, S), bf16, "cd16")
            cs16 = load(cs16_d, (P, S), bf16, "cs16")
            wqk16 = [load(wqk16_d[c], (P, P), bf16, f"wqk{c}") for c in range(DC)]
            wv16 = [load(wv16_d[c], (P, HD), bf16, f"wv{c}") for c in range(DC)]
            wo16 = [load(wo16_d[c], (P, Dm), bf16, f"wo{c}") for c in range(DC)]
            mask16 = [
                load(mask16_d[i], (P, mcols), bf16, f"mask{i}") for i in range(TCH)
            ]
            g1c = [load(g1_d[c], (P, 1), f32, f"g1{c}") for c in range(DC)]
            b1c = [load(b1_d[c], (P, 1), f32, f"b1{c}") for c in range(DC)]
            g2c = [load(g2_d[c], (P, 1), f32, f"g2{c}") for c in range(DC)]
            b2c = [load(b2_d[c], (P, 1), f32, f"b2{c}") for c in range(DC)]
            gw32 = [load(gw32_d[c], (P, E), f32, f"gw{c}") for c in range(DC)]
            gb32 = load(gb_d, (P, TCH, E), f32, "gb32")
            ones16 = load(ones16_d, (1, P), bf16, "ones16")
            sel32 = load(sel_d, (P, 1), f32, "sel32")
            id128 = load(id128_d, (P, P), f32, "id128")
            ek16 = [
                [load(ek16_d[e, c], (P, FF), bf16, f"ek{e}_{c}") for c in range(DC)]
                for e in range(2)
            ]
            ev16 = [
                [load(ev16_d[e, c], (P, Dm), bf16, f"ev{e}_{c}") for c in range(FCH)]
                for e in range(2)
            ]
            sk16 = [load(sk16_d[c], (P, SHF), bf16, f"sk{c}") for c in range(DC)]
            sv16 = load(sv16_d, (SHF, Dm), bf16, "sv16")

            # ---------- phase 1: dyt1 + per-head attention ----------
            xT32 = []
            hT16 = []
            for c in range(DC):
                xt = cst.tile((P, S), f32, name=f"xT{c}", tag=f"xT{c}")
                nc.sync.dma_start(xt[:], xT32_d[c])
                xT32.append(xt)
                th = tmp.tile((P, S), f32, name="th", tag="t32")
                nc.scalar.activation(th[:], xt[:], Act.Tanh, scale=float(a1v))
                u = tmp.tile((P, S), f32, name="u", tag="t32")
                nc.vector.tensor_tensor(
                    u[:], th[:], g1c[c][:].to_broadcast((P, S)), Alu.mult
                )
                ht = cst.tile((P, S), bf16, name=f"hT16_{c}", tag=f"hT16_{c}")
                nc.vector.tensor_tensor(
                    ht[:], u[:], b1c[c][:].to_broadcast((P, S)), Alu.add
                )
                hT16.append(ht)

            # qkT = [wq*0.125 | wk]^T @ h  -> [128 (q64|k64), S]
            qk_ps = psp.tile((P, S), f32, name="qk_ps", tag="mm")
            for c in range(DC):
                nc.tensor.matmul(
                    qk_ps[:], lhsT=wqk16[c][:], rhs=hT16[c][:],
                    start=(c == 0), stop=(c == DC - 1),
                )

            # v (untransposed): [t-chunk][128, 64]
            v16 = []
            for t in range(TCH):
                v_ps = psp.tile((P, HD), f32, name="v_ps", tag="mm")
                for c in range(DC):
                    nc.tensor.matmul(
                        v_ps[:], lhsT=hT16[c][:, ts(t, P)], rhs=wv16[c][:],
                        start=(c == 0), stop=(c == DC - 1),
                    )
                vt = cst.tile((P, HD), bf16, name=f"v16_{t}", tag=f"v16_{t}")
                nc.vector.tensor_copy(vt[:], v_ps[:])
                v16.append(vt)

            # rope on packed qk
            r1 = tmp.tile((P, S), f32, name="r1", tag="t32")
            nc.vector.tensor_tensor(r1[:], qk_ps[:], cd16[:], Alu.mult)
            sw = tmp.tile((P, S), f32, name="sw", tag="t32")
            half = HD // 2  # 32
            swap_src = [1, 0, 3, 2]  # 32-row block read for each output block
            for b in range(4):
                nc.vector.tensor_tensor(
                    sw[b * half:(b + 1) * half, :],
                    qk_ps[swap_src[b] * half:(swap_src[b] + 1) * half, :],
                    cs16[b * half:(b + 1) * half, :],
                    Alu.mult,
                )
            qrot = cst.tile((HD, S), bf16, name="qrot", tag="qrot")
            nc.vector.tensor_tensor(qrot[:], r1[0:HD, :], sw[0:HD, :], Alu.add)
            krot = cst.tile((HD, S), bf16, name="krot", tag="krot")
            nc.vector.tensor_tensor(krot[:], r1[HD:P, :], sw[HD:P, :], Alu.add)

            # scores/softmax/attn per query chunk, causal-lower-triangle only
            avT_ps = psp.tile((HD, S), f32, name="avT_ps", tag="avT", bufs=1)
            for i in range(TCH):
                L = P * (i + 1)
                sc_ps = psp.tile((P, S), f32, name="sc_ps", tag="mm")
                nc.tensor.matmul(
                    sc_ps[:, :L],
                    lhsT=qrot[:, ts(i, P)],
                    rhs=krot[:, 0:L],
                    start=True, stop=True,
                )
                if full_mask:
                    nc.vector.tensor_tensor(
                        sc_ps[:, :L], sc_ps[:, :L], mask16[i][:, :L], Alu.add
                    )
                else:
                    nc.vector.tensor_tensor(
                        sc_ps[:, ts(i, P)], sc_ps[:, ts(i, P)], mask16[i][:],
                        Alu.add,
                    )
                negmax = tmp.tile((P, 1), f32, name="negmax", tag="red")
                nc.vector.reduce_max(negmax[:], sc_ps[:, :L], axis=AX.X,
                                     negate=True)
                e32 = tmp.tile((P, S), f32, name="e32", tag="t32")
                nc.scalar.activation(e32[:, :L], sc_ps[:, :L], Act.Exp,
                                     bias=negmax[:], scale=1.0)
                ssum = tmp.tile((P, 1), f32, name="ssum", tag="red")
                nc.vector.reduce_sum(ssum[:], e32[:, :L], axis=AX.X)
                rinv = tmp.tile((P, 1), f32, name="rinv", tag="red")
                nc.vector.reciprocal(rinv[:], ssum[:])
                pr16 = tmp.tile((P, S), bf16, name="pr16", tag="pr16")
                nc.vector.tensor_tensor(
                    pr16[:, :L], e32[:, :L], rinv[:].to_broadcast((P, L)),
                    Alu.mult,
                )
                for j in range(i + 1):
                    at = tmp.tile((P, P), bf16, name="at", tag="at", bufs=6)
                    nc.sync.dma_start(at[:], pr16[:, ts(j, P)], transpose=True)
                    nc.tensor.matmul(
                        avT_ps[:, ts(i, P)], lhsT=v16[j][:], rhs=at[:],
                        start=(j == 0), stop=(j == i),
                    )

            ao16 = cst.tile((HD, S), bf16, name="ao16", tag="ao16")
            nc.vector.tensor_copy(ao16[:], avT_ps[:])

            # ---------- AllGather attention outputs (heads) ----------
            ag_in = drp.tile((HD, S), bf16, name="ag_in")
            ag_out = drp.tile((H * HD, S), bf16, name="ag_out",
                              addr_space="Shared")
            nc.sync.dma_start(ag_in[:], ao16[:])
            nc.gpsimd.collective_compute(
                "AllGather",
                Alu.bypass,
                replica_groups=[list(range(NCORES))],
                ins=[ag_in[:]],
                outs=[ag_out[:]],
            )
            aoT16 = []
            for c in range(DC):
                t = cst.tile((P, S), bf16, name=f"aoT16_{c}", tag=f"aoT16_{c}")
                nc.sync.dma_start(t[:], ag_out[ts(c, P), :])
                aoT16.append(t)

            # ---------- wo projection + residual + dyt2 ----------
            x1T32 = []
            h2T32 = []
            h2T16 = []
            for m in range(DC):
                pw = psp.tile((P, S), f32, name="pw", tag="mm")
                for k in range(DC):
                    nc.tensor.matmul(
                        pw[:], lhsT=wo16[k][:, ts(m, P)], rhs=aoT16[k][:],
                        start=(k == 0), stop=(k == DC - 1),
                    )
                x1 = cst.tile((P, S), f32, name=f"x1T{m}", tag=f"x1T{m}")
                nc.vector.tensor_tensor(x1[:], pw[:], xT32[m][:], Alu.add)
                x1T32.append(x1)
                th = tmp.tile((P, S), f32, name="th2", tag="t32")
                nc.scalar.activation(th[:], x1[:], Act.Tanh, scale=float(a2v))
                u = tmp.tile((P, S), f32, name="u2", tag="t32")
                nc.vector.tensor_tensor(
                    u[:], th[:], g2c[m][:].to_broadcast((P, S)), Alu.mult
                )
                h2 = cst.tile((P, S), f32, name=f"h2T32_{m}", tag=f"h2T32_{m}")
                nc.vector.tensor_tensor(
                    h2[:], u[:], b2c[m][:].to_broadcast((P, S)), Alu.add
                )
                h2T32.append(h2)
                h216 = cst.tile((P, S), bf16, name=f"h2T16_{m}", tag=f"h2T16_{m}")
                nc.vector.tensor_copy(h216[:], h2[:])
                h2T16.append(h216)

            # ---------- router (fp32) + top-2 gates ----------
            lg_ps = psp.tile((P, TCH, E), f32, name="lg_ps", tag="lg", bufs=1)
            for t in range(TCH):
                for c in range(DC):
                    nc.tensor.matmul(
                        lg_ps[:, t, :],
                        lhsT=h2T32[c][:, ts(t, P)],
                        rhs=gw32[c][:],
                        start=(c == 0), stop=(c == DC - 1),
                    )
            lg32 = cst.tile((P, TCH, E), f32, name="lg32", tag="lg32")
            nc.vector.tensor_tensor(lg32[:], lg_ps[:], gb32[:], Alu.add)
            ex32 = cst.tile((P, TCH, E), f32, name="ex32", tag="ex32")
            nc.scalar.activation(ex32[:], lg32[:], Act.Exp, scale=1.0)
            ssum4 = cst.tile((P, TCH), f32, name="ssum4", tag="ssum4")
            nc.vector.reduce_sum(ssum4[:], ex32[:], axis=AX.X)
            rinv4 = cst.tile((P, TCH), f32, name="rinv4", tag="rinv4")
            nc.vector.reciprocal(rinv4[:], ssum4[:])
            prb = cst.tile((P, TCH, E), f32, name="prb", tag="prb")
            nc.vector.tensor_tensor(
                prb[:], ex32[:], rinv4[:, :, None].to_broadcast((P, TCH, E)),
                Alu.mult,
            )
            m1 = cst.tile((P, TCH), f32, name="m1", tag="m1")
            nc.vector.reduce_max(m1[:], prb[:], axis=AX.X)
            ge1 = cst.tile((P, TCH, E), f32, name="ge1", tag="ge1")
            nc.vector.tensor_tensor(
                ge1[:], prb[:], m1[:, :, None].to_broadcast((P, TCH, E)),
                Alu.is_ge,
            )
            msk = cst.tile((P, TCH, E), f32, name="msk", tag="msk")
            nc.vector.scalar_tensor_tensor(
                msk[:], ge1[:], -1e9, prb[:], op0=Alu.mult, op1=Alu.add
            )
            m2 = cst.tile((P, TCH), f32, name="m2", tag="m2")
            nc.vector.reduce_max(m2[:], msk[:], axis=AX.X)
            ge2 = cst.tile((P, TCH, E), f32, name="ge2", tag="ge2")
            nc.vector.tensor_tensor(
                ge2[:], prb[:], m2[:, :, None].to_broadcast((P, TCH, E)),
                Alu.is_ge,
            )
            wg = cst.tile((P, TCH, E), f32, name="wg", tag="wg")
            nc.vector.tensor_tensor(wg[:], prb[:], ge2[:], Alu.mult)

            # transpose the two local experts' gate columns ([128,1] -> [1,128]
            # each, so every row lands at partition base 0)
            wrow = [
                cst.tile((1, S), bf16, name=f"wrow{el}", tag=f"wrow{el}")
                for el in range(2)
            ]
            for t in range(TCH):
                for el in range(2):
                    wt_ps = psp.tile((1, P), f32, name="wt_ps", tag="avT",
                                     bufs=1)
                    nc.tensor.transpose(wt_ps[:], wg[:, t, el:el + 1], id128[:])
                    nc.vector.tensor_copy(wrow[el][:, ts(t, P)], wt_ps[:])

            # broadcast local-expert gate rows across partitions (rank-1 mm)
            rep16 = []
            for el in range(2):
                rp_ps = psp.tile((P, S), f32, name="rp_ps", tag="mm")
                nc.tensor.matmul(
                    rp_ps[:], lhsT=ones16[:], rhs=wrow[el][:],
                    start=True, stop=True,
                )
                rp = cst.tile((P, S), bf16, name=f"rep16_{el}", tag=f"rep16_{el}")
                nc.vector.tensor_copy(rp[:], rp_ps[:])
                rep16.append(rp)

            # ---------- experts (dense, gated) + shared expert ----------
            moe_ps = [
                psp.tile((P, S), f32, name=f"moe_ps{m}", tag="moe", bufs=4)
                for m in range(DC)
            ]

            def gelu_gate(src_ps, rows, gate_rep):
                """gelu(u) [* gate], u = src_ps[:rows]; returns bf16 tile."""
                u16 = tmp.tile((P, S), bf16, name="u16", tag="u16", bufs=3)
                nc.vector.tensor_copy(u16[:rows], src_ps[:rows])
                x2 = tmp.tile((P, S), bf16, name="x2", tag="x2", bufs=3)
                nc.vector.tensor_tensor(x2[:rows], u16[:rows], u16[:rows],
                                        Alu.mult)
                t1 = tmp.tile((P, S), bf16, name="t1", tag="x2", bufs=3)
                nc.vector.tensor_scalar(
                    t1[:rows], x2[:rows], GELU_A, 1.0,
                    op0=Alu.mult, op1=Alu.add,
                )
                mm_ = tmp.tile((P, S), bf16, name="mm_", tag="x2", bufs=3)
                nc.vector.tensor_tensor(mm_[:rows], u16[:rows], t1[:rows],
                                        Alu.mult)
                sg = tmp.tile((P, S), bf16, name="sg", tag="x2", bufs=3)
                nc.scalar.activation(sg[:rows], mm_[:rows], Act.Sigmoid,
                                     scale=GELU_C)
                if gate_rep is not None:
                    nc.vector.tensor_tensor(sg[:rows], sg[:rows],
                                            gate_rep[:rows], Alu.mult)
                g = tmp.tile((P, S), bf16, name="g", tag="g16", bufs=6)
                nc.vector.tensor_tensor(g[:rows], u16[:rows], sg[:rows],
                                        Alu.mult)
                return g

            first_mm = True
            for el in range(2):
                g16 = []
                for fc in range(FCH):
                    up_ps = psp.tile((P, S), f32, name="up_ps", tag="mm")
                    for c in range(DC):
                        nc.tensor.matmul(
                            up_ps[:],
                            lhsT=ek16[el][c][:, ts(fc, P)],
                            rhs=h2T16[c][:],
                            start=(c == 0), stop=(c == DC - 1),
                        )
                    g16.append(gelu_gate(up_ps, P, rep16[el]))
                for m in range(DC):
                    for fc in range(FCH):
                        nc.tensor.matmul(
                            moe_ps[m][:],
                            lhsT=ev16[el][fc][:, ts(m, P)],
                            rhs=g16[fc][:, :],
                            start=(el == 0 and fc == 0), stop=False,
                        )

            su_ps = psp.tile((SHF, S), f32, name="su_ps", tag="mm")
            for c in range(DC):
                nc.tensor.matmul(
                    su_ps[:], lhsT=sk16[c][:], rhs=h2T16[c][:],
                    start=(c == 0), stop=(c == DC - 1),
                )
            gs16 = gelu_gate(su_ps, SHF, None)
            for m in range(DC):
                nc.tensor.matmul(
                    moe_ps[m][:], lhsT=sv16[:, ts(m, P)], rhs=gs16[:SHF, :],
                    start=False, stop=True,
                )

            # ---------- fold residual (core 0) + ReduceScatter ----------
            rs_in = drp.tile((Dm, S), f32, name="rs_in")
            rs_out = drp.tile((DSH, S), f32, name="rs_out")
            for m in range(DC):
                fin = tmp.tile((P, S), f32, name="fin", tag="t32")
                nc.vector.scalar_tensor_tensor(
                    fin[:], x1T32[m][:], sel32[:], moe_ps[m][:],
                    op0=Alu.mult, op1=Alu.add,
                )
                nc.sync.dma_start(rs_in[ts(m, P), :], fin[:])
            nc.gpsimd.collective_compute(
                "ReduceScatter",
                Alu.add,
                replica_groups=[list(range(NCORES))],
                ins=[rs_in[:]],
                outs=[rs_out[:]],
            )
            nc.sync.dma_start(outT_d[:], rs_out[:])

    nc.compile()
    return nc


def _prep_inputs(inputs):
    """Host-side sharding/layout prep. Returns (in_maps, a1v, a2v, full_mask)."""
    x = np.asarray(inputs["x"], np.float32)            # [1,S,D]
    attn_mask = np.asarray(inputs["attn_mask"])        # [1,S]
    wq = np.asarray(inputs["wq"], np.float32)
    wk = np.asarray(inputs["wk"], np.float32)
    wv = np.asarray(inputs["wv"], np.float32)
    wo = np.asarray(inputs["wo"], np.float32)
    a1 = float(np.asarray(inputs["a1"]).reshape(-1)[0])
    g1 = np.asarray(inputs["g1"], np.float32).reshape(Dm)
    b1 = np.asarray(inputs["b1"], np.float32).reshape(Dm)
    a2 = float(np.asarray(inputs["a2"]).reshape(-1)[0])
    g2 = np.asarray(inputs["g2"], np.float32).reshape(Dm)
    b2 = np.asarray(inputs["b2"], np.float32).reshape(Dm)
    gate_w = np.asarray(inputs["gate_w"], np.float32)  # [D,E]
    gate_b = np.asarray(inputs["gate_b"], np.float32).reshape(E)
    ek = np.asarray(inputs["ek"], np.float32)          # [E,D,FF]
    ev = np.asarray(inputs["ev"], np.float32)          # [E,FF,D]
    sk = np.asarray(inputs["sk"], np.float32)          # [1,D,FF]
    sv = np.asarray(inputs["sv"], np.float32)          # [1,FF,D]

    xT = np.ascontiguousarray(x[0].T)                  # [D,S]
    xT32 = xT.reshape(DC, P, S)

    # rope tables (transposed layout: [freq, pos])
    pos = np.arange(S, dtype=np.float32)
    half = HD // 2
    inv = 1.0 / (10000.0 ** (np.arange(half, dtype=np.float32) / half))
    ang = pos[:, None] * inv[None, :]                  # [S, half]
    cosT = np.cos(ang).T.astype(np.float32)            # [32,S]
    sinT = np.sin(ang).T.astype(np.float32)
    cd16 = np.concatenate([cosT, cosT, cosT, cosT], 0).astype(BF)
    cs16 = np.concatenate([-sinT, sinT, -sinT, sinT], 0).astype(BF)

    # additive attention mask, exactly as the reference builds it
    causal = np.tril(np.ones((S, S), np.float32))
    am = attn_mask.astype(np.float32)[0]               # [S]
    cm = causal * am[None, :]
    cm[np.arange(S), np.arange(S)] = 1.0
    addmask = -(1.0 - cm) * 1e9                        # [S,S]
    offdiag_needed = False
    for i in range(TCH):
        blk = addmask[ts_ := slice(i * P, (i + 1) * P), : i * P]
        if blk.size and np.any(blk != 0.0):
            offdiag_needed = True
    full_mask = bool(offdiag_needed)
    if full_mask:
        mask16 = np.stack(
            [addmask[i * P:(i + 1) * P, :] for i in range(TCH)]
        ).astype(BF)
    else:
        mask16 = np.stack(
            [addmask[i * P:(i + 1) * P, i * P:(i + 1) * P] for i in range(TCH)]
        ).astype(BF)

    wo16 = wo.reshape(DC, P, Dm).astype(BF)
    id128 = np.eye(P, dtype=np.float32)
    ones16 = np.ones((1, P), BF)
    gb_base = gate_b  # permuted per core below

    common = dict(
        xT32=xT32, cd16=cd16, cs16=cs16, mask16=mask16,
        g1c=g1.reshape(DC, P, 1), b1c=b1.reshape(DC, P, 1),
        g2c=g2.reshape(DC, P, 1), b2c=b2.reshape(DC, P, 1),
        wo16=wo16, ones16=ones16, id128=id128,
    )

    in_maps = []
    for c in range(NCORES):
        hsl = slice(c * HD, (c + 1) * HD)
        wqk = np.concatenate([wq[:, hsl] * 0.125, wk[:, hsl]], axis=1)
        perm = [2 * c, 2 * c + 1] + [e for e in range(E)
                                     if e not in (2 * c, 2 * c + 1)]
        gwp = gate_w[:, perm]
        gbp = gb_base[perm]
        m = dict(common)
        m.update(
            wqk16=wqk.reshape(DC, P, P).astype(BF),
            wv16=wv[:, hsl].reshape(DC, P, HD).astype(BF),
            gw32=gwp.reshape(DC, P, E).astype(np.float32),
            gb=np.broadcast_to(gbp, (P, TCH, E)).astype(np.float32).copy(),
            sel=np.full((P, 1), 1.0 if c == 0 else 0.0, np.float32),
            ek16=np.stack([ek[2 * c + e].reshape(DC, P, FF)
                           for e in range(2)]).astype(BF),
            ev16=np.stack([ev[2 * c + e].reshape(FCH, P, Dm)
                           for e in range(2)]).astype(BF),
            sk16=sk[0][:, c * SHF:(c + 1) * SHF].reshape(DC, P, SHF).astype(BF),
            sv16=np.ascontiguousarray(
                sv[0][c * SHF:(c + 1) * SHF, :]).astype(BF),
        )
        in_maps.append(m)
    return in_maps, a1, a2, full_mask


def kernel(**inputs):
    from concourse import bass_utils

    in_maps, a1v, a2v, full_mask = _prep_inputs(inputs)
    key = (a1v, a2v, full_mask)
    if key not in _PROG_CACHE:
        _PROG_CACHE[key] = _build_program(a1v, a2v, full_mask)
    nc = _PROG_CACHE[key]

    if os.environ.get("BASSK_SIM"):
        from concourse.bass_interp import MultiCoreSim

        sim = MultiCoreSim(nc, num_cores=NCORES)
        for c in range(NCORES):
            for k, v in in_maps[c].items():
                sim.cores[c].tensor(k)[:] = v
        sim.simulate(check_with_hw=False)
        shards = [np.array(sim.cores[c].tensor("outT")) for c in range(NCORES)]
    else:
        trace = bool(os.environ.get("BASSK_TRACE"))
        res = bass_utils.run_bass_kernel_spmd(
            nc, in_maps, core_ids=list(range(NCORES)), trace=trace
        )
        LAST_INFO["exec_time_ns"] = res.exec_time_ns
        LAST_INFO["profile_json"] = res.profile_json
        shards = [np.asarray(res.results[c]["outT"]) for c in range(NCORES)]

    outT = np.concatenate(shards, axis=0)              # [D,S]
    return np.ascontiguousarray(outT.T).reshape(1, S, Dm).astype(np.float32)


# revision 8
# speedup vs baseline: 1.2660x; 1.2660x over previous
"""Trainium2 Bass kernel for a transformer block: DyT-prenorm attention (RoPE,
causal+mask) + top-2-of-16 MoE with a shared expert.

Strategy (8 NeuronCores, SPMD single program, per-core data):
  * Attention head-parallel: core c computes head c (QK packed in one lhsT,
    rope via two elementwise passes), produces avT [64, S] and AllGathers it
    (bf16) so every core can run the wo projection + residual + dyt locally.
  * Router replicated (fp32 for top-2 selection stability); gate_w columns are
    permuted per core so the local experts are always columns 0 and 1.
  * Experts expert-parallel: 2 dense experts per core (gate weights folded in
    via a rank-1 broadcast matmul), shared expert sharded over FF (64 cols per
    core); everything accumulates into 4 PSUM tiles which are ReduceScattered
    (fp32) across cores; core 0 also contributes x + attention (residual).
  * All big matmuls bf16 with fp32 PSUM accumulation.
  * All constants/weights are packed host-side into a few [128, C] arrays so
    the whole load phase is 5 DMA dispatches (each fans out over the 16 HW
    DMA engines).
Everything is computed transposed (d on partitions, tokens on the free axis);
the host transposes the output back.
"""

import os
import numpy as np
import ml_dtypes

BF = ml_dtypes.bfloat16

S = 512      # tokens (B=1)
Dm = 512     # d_model
H = 8        # heads
HD = 64      # head dim
E = 16       # experts
FF = 512     # expert hidden
P = 128
NCORES = 8
DC = Dm // P    # 4 d-model chunks
TCH = S // P    # 4 token chunks
FCH = FF // P   # 4 ff chunks
SHF = FF // NCORES  # shared-expert ff slice per core (64)
DSH = Dm // NCORES  # output row shard per core (64)

GELU_C = float(2.0 * np.sqrt(2.0 / np.pi))  # sigmoid-form tanh-gelu scale
GELU_A = 0.044715

_PROG_CACHE = {}

LAST_INFO = {}


def _layouts(full_mask):
    """Column layouts of the packed constant arrays (shared host/device)."""
    def lay(blocks):
        off, out = 0, {}
        for name, cols in blocks:
            out[name] = (off, cols)
            off += cols
        return out, off

    atn, atn_c = lay([
        ("wqk", DC * P), ("wv", DC * HD), ("cd", S), ("cs", S),
        ("mask", TCH * (S if full_mask else P)), ("idbf", P), ("ones", P),
    ])
    mid, mid_c = lay([("wo", DC * Dm)])
    moe, moe_c = lay([
        ("ek", 2 * DC * FF), ("ev", 2 * FCH * Dm), ("sk", DC * SHF),
        ("sv", Dm),
    ])
    p32, p32_c = lay([
        ("g1", DC), ("b1", DC), ("g2", DC), ("b2", DC),
        ("gw", DC * E), ("gb", TCH * E), ("sel", 1), ("idf", P),
    ])
    return (atn, atn_c), (mid, mid_c), (moe, moe_c), (p32, p32_c)


def _build_program(a1v: float, a2v: float, full_mask: bool, sim_gelu: bool):
    import concourse.bass as bass
    import concourse.mybir as mybir
    import concourse.tile as tile
    from concourse import bacc

    f32 = mybir.dt.float32
    bf16 = mybir.dt.bfloat16
    Alu = mybir.AluOpType
    Act = mybir.ActivationFunctionType
    AX = mybir.AxisListType
    ts = bass.ts

    (atn_l, atn_c), (mid_l, mid_c), (moe_l, moe_c), (p32_l, p32_c) = \
        _layouts(full_mask)

    nc = bacc.Bacc(
        "TRN2", target_bir_lowering=False, debug=False, num_devices=NCORES
    )

    def inp(name, shape, dt=f32):
        return nc.dram_tensor(name, list(shape), dt, kind="ExternalInput").ap()

    xT_d = inp("xT", (P, DC * S))
    atn_d = inp("atn16", (P, atn_c), bf16)
    mid_d = inp("mid16", (P, mid_c), bf16)
    moe_d = inp("moe16", (P, moe_c), bf16)
    p32_d = inp("p32", (P, p32_c))

    outT_d = nc.dram_tensor("outT", [DSH, S], f32, kind="ExternalOutput").ap()

    with tile.TileContext(nc, num_cores=NCORES) as tc:
        with (
            tc.tile_pool(name="cst", bufs=1) as cst,
            tc.tile_pool(name="tmp", bufs=3) as tmp,
            tc.tile_pool(name="ps", bufs=2, space="PSUM") as psp,
            tc.tile_pool(name="dram", bufs=1, space="DRAM") as drp,
        ):
            # ---------- warmup collective (hides ncfw first-use latency) ----
            wa_in = drp.tile((1, 16), bf16, name="wa_in")
            wa_out = drp.tile((8, 16), bf16, name="wa_out", addr_space="Shared")
            wa_sb = cst.tile((1, 16), bf16, name="wa_sb", tag="wa_sb")
            nc.gpsimd.dma_start(wa_in[:], atn_d[0:1, 0:16])
            nc.gpsimd.collective_compute(
                "AllGather", Alu.bypass,
                replica_groups=[list(range(NCORES))],
                ins=[wa_in[:]], outs=[wa_out[:]],
            )
            nc.gpsimd.dma_start(wa_sb[:], wa_out[0:1, :])

            # ---------- packed loads (5 DMA dispatches, 2 queues) ----------
            xTt = cst.tile((P, DC * S), f32, name="xTt", tag="xTt")
            nc.sync.dma_start(xTt[:], xT_d[:])
            pk32 = cst.tile((P, p32_c), f32, name="pk32", tag="pk32")
            nc.scalar.dma_start(pk32[:], p32_d[:])
            atn16 = cst.tile((P, atn_c), bf16, name="atn16", tag="atn16")
            nc.sync.dma_start(atn16[:], atn_d[:])
            mid16 = cst.tile((P, mid_c), bf16, name="mid16", tag="mid16")
            nc.scalar.dma_start(mid16[:], mid_d[:])
            moe16 = cst.tile((P, moe_c), bf16, name="moe16", tag="moe16")
            nc.sync.dma_start(moe16[:], moe_d[:])

            def asl(name, c=0, w=None):  # attention-pack slice
                off, cols = atn_l[name]
                w = cols if w is None else w
                return atn16[:, off + c * w: off + (c + 1) * w]

            def psl(name, c=0, w=None):  # fp32-pack slice
                off, cols = p32_l[name]
                w = cols if w is None else w
                return pk32[:, off + c * w: off + (c + 1) * w]

            def msl(name, c=0, w=None):  # moe-pack slice
                off, cols = moe_l[name]
                w = cols if w is None else w
                return moe16[:, off + c * w: off + (c + 1) * w]

            idbf = asl("idbf")
            ones16 = asl("ones")[0:1, :]
            idf = psl("idf")
            sel32 = psl("sel")

            # ---------- phase 1: dyt1 + per-head attention ----------
            hT16 = []
            for c in range(DC):
                th = tmp.tile((P, S), f32, name="th", tag="t32")
                nc.scalar.activation(th[:], xTt[:, ts(c, S)], Act.Tanh,
                                     scale=float(a1v))
                ht = cst.tile((P, S), bf16, name=f"hT16_{c}", tag=f"hT16_{c}")
                nc.vector.scalar_tensor_tensor(
                    ht[:], th[:], psl("g1", c, 1),
                    psl("b1", c, 1).to_broadcast((P, S)),
                    op0=Alu.mult, op1=Alu.add,
                )
                hT16.append(ht)

            # qkT = [wq*0.125 | wk]^T @ h  -> [128 (q64|k64), S]
            qk_ps = psp.tile((P, S), f32, name="qk_ps", tag="mm")
            for c in range(DC):
                nc.tensor.matmul(
                    qk_ps[:], lhsT=asl("wqk", c, P), rhs=hT16[c][:],
                    start=(c == 0), stop=(c == DC - 1),
                )

            # v (untransposed): [t-chunk][128, 64]
            v16 = []
            for t in range(TCH):
                v_ps = psp.tile((P, HD), f32, name="v_ps", tag="mm")
                for c in range(DC):
                    nc.tensor.matmul(
                        v_ps[:], lhsT=hT16[c][:, ts(t, P)], rhs=asl("wv", c, HD),
                        start=(c == 0), stop=(c == DC - 1),
                    )
                vt = cst.tile((P, HD), bf16, name=f"v16_{t}", tag=f"v16_{t}")
                nc.vector.tensor_copy(vt[:], v_ps[:])
                v16.append(vt)

            # rope on packed qk
            r1 = tmp.tile((P, S), f32, name="r1", tag="t32")
            nc.vector.tensor_tensor(r1[:], qk_ps[:], asl("cd"), Alu.mult)
            sw = tmp.tile((P, S), f32, name="sw", tag="t32")
            half = HD // 2  # 32
            swap_src = [1, 0, 3, 2]  # 32-row block read for each output block
            cs_ap = asl("cs")
            for b in range(4):
                nc.vector.tensor_tensor(
                    sw[b * half:(b + 1) * half, :],
                    qk_ps[swap_src[b] * half:(swap_src[b] + 1) * half, :],
                    cs_ap[b * half:(b + 1) * half, :],
                    Alu.mult,
                )
            qrot = cst.tile((HD, S), bf16, name="qrot", tag="qrot")
            nc.vector.tensor_tensor(qrot[:], r1[0:HD, :], sw[0:HD, :], Alu.add)
            krot = cst.tile((HD, S), bf16, name="krot", tag="krot")
            nc.vector.tensor_tensor(krot[:], r1[HD:P, :], sw[HD:P, :], Alu.add)

            # scores/softmax/attn per query chunk, causal-lower-triangle only
            mw = S if full_mask else P
            avT_ps = psp.tile((HD, S), f32, name="avT_ps", tag="avT", bufs=1)
            for i in range(TCH):
                L = P * (i + 1)
                sc_ps = psp.tile((P, S), f32, name="sc_ps", tag="mm")
                nc.tensor.matmul(
                    sc_ps[:, :L], lhsT=qrot[:, ts(i, P)], rhs=krot[:, 0:L],
                    start=True, stop=True,
                )
                if full_mask:
                    nc.vector.tensor_tensor(
                        sc_ps[:, :L], sc_ps[:, :L], asl("mask", i, mw)[:, :L],
                        Alu.add,
                    )
                else:
                    nc.vector.tensor_tensor(
                        sc_ps[:, ts(i, P)], sc_ps[:, ts(i, P)],
                        asl("mask", i, mw), Alu.add,
                    )
                negmax = tmp.tile((P, 1), f32, name="negmax", tag="red")
                nc.vector.reduce_max(negmax[:], sc_ps[:, :L], axis=AX.X,
                                     negate=True)
                e32 = tmp.tile((P, S), f32, name="e32", tag="t32")
                nc.scalar.activation(e32[:, :L], sc_ps[:, :L], Act.Exp,
                                     bias=negmax[:], scale=1.0)
                ssum = tmp.tile((P, 1), f32, name="ssum", tag="red")
                nc.vector.reduce_sum(ssum[:], e32[:, :L], axis=AX.X)
                rinv = tmp.tile((P, 1), f32, name="rinv", tag="red")
                nc.vector.reciprocal(rinv[:], ssum[:])
                pr16 = tmp.tile((P, S), bf16, name="pr16", tag="pr16")
                nc.vector.tensor_tensor(
                    pr16[:, :L], e32[:, :L], rinv[:].to_broadcast((P, L)),
                    Alu.mult,
                )
                for j in range(i + 1):
                    at_ps = psp.tile((P, P), bf16, name="at_ps", tag="lg",
                                     bufs=1)
                    nc.tensor.transpose(at_ps[:], pr16[:, ts(j, P)], idbf)
                    at = tmp.tile((P, P), bf16, name="at", tag="at", bufs=4)
                    nc.vector.tensor_copy(at[:], at_ps[:])
                    nc.tensor.matmul(
                        avT_ps[:, ts(i, P)], lhsT=v16[j][:], rhs=at[:],
                        start=(j == 0), stop=(j == i),
                    )

            ao16 = cst.tile((HD, S), bf16, name="ao16", tag="ao16")
            nc.vector.tensor_copy(ao16[:], avT_ps[:])

            # ---------- AllGather attention outputs (heads) ----------
            ag_in = drp.tile((HD, S), bf16, name="ag_in")
            ag_out = drp.tile((H * HD, S), bf16, name="ag_out",
                              addr_space="Shared")
            nc.sync.dma_start(ag_in[:], ao16[:])
            nc.gpsimd.collective_compute(
                "AllGather", Alu.bypass,
                replica_groups=[list(range(NCORES))],
                ins=[ag_in[:]], outs=[ag_out[:]],
            )
            aoT16 = []
            for c in range(DC):
                t = cst.tile((P, S), bf16, name=f"aoT16_{c}", tag=f"aoT16_{c}")
                nc.sync.dma_start(t[:], ag_out[ts(c, P), :])
                aoT16.append(t)

            # ---------- wo projection + residual + dyt2 ----------
            x1T32 = []
            h2T32 = []
            h2T16 = []
            for m in range(DC):
                pw = psp.tile((P, S), f32, name="pw", tag="mm")
                for k in range(DC):
                    nc.tensor.matmul(
                        pw[:], lhsT=mid16[:, mid_l["wo"][0] + k * Dm + m * P:
                                         mid_l["wo"][0] + k * Dm + (m + 1) * P],
                        rhs=aoT16[k][:],
                        start=(k == 0), stop=(k == DC - 1),
                    )
                x1 = cst.tile((P, S), f32, name=f"x1T{m}", tag=f"x1T{m}")
                nc.vector.tensor_tensor(x1[:], pw[:], xTt[:, ts(m, S)], Alu.add)
                x1T32.append(x1)
                th = tmp.tile((P, S), f32, name="th2", tag="t32")
                nc.scalar.activation(th[:], x1[:], Act.Tanh, scale=float(a2v))
                h2 = cst.tile((P, S), f32, name=f"h2T32_{m}", tag=f"h2T32_{m}")
                nc.vector.scalar_tensor_tensor(
                    h2[:], th[:], psl("g2", m, 1),
                    psl("b2", m, 1).to_broadcast((P, S)),
                    op0=Alu.mult, op1=Alu.add,
                )
                h2T32.append(h2)
                h216 = cst.tile((P, S), bf16, name=f"h2T16_{m}", tag=f"h2T16_{m}")
                nc.vector.tensor_copy(h216[:], h2[:])
                h2T16.append(h216)

            # ---------- router (fp32) + top-2 gates ----------
            lg_ps = psp.tile((P, TCH, E), f32, name="lg_ps", tag="lg", bufs=1)
            for t in range(TCH):
                for c in range(DC):
                    nc.tensor.matmul(
                        lg_ps[:, t, :], lhsT=h2T32[c][:, ts(t, P)],
                        rhs=psl("gw", c, E),
                        start=(c == 0), stop=(c == DC - 1),
                    )
            gb_ap = psl("gb").rearrange("p (t e) -> p t e", e=E)
            lg32 = cst.tile((P, TCH, E), f32, name="lg32", tag="lg32")
            nc.vector.tensor_tensor(lg32[:], lg_ps[:], gb_ap, Alu.add)
            ex32 = cst.tile((P, TCH, E), f32, name="ex32", tag="ex32")
            nc.scalar.activation(ex32[:], lg32[:], Act.Exp, scale=1.0)
            ssum4 = cst.tile((P, TCH), f32, name="ssum4", tag="ssum4")
            nc.vector.reduce_sum(ssum4[:], ex32[:], axis=AX.X)
            rinv4 = cst.tile((P, TCH), f32, name="rinv4", tag="rinv4")
            nc.vector.reciprocal(rinv4[:], ssum4[:])
            prb = cst.tile((P, TCH, E), f32, name="prb", tag="prb")
            nc.vector.tensor_tensor(
                prb[:], ex32[:], rinv4[:, :, None].to_broadcast((P, TCH, E)),
                Alu.mult,
            )
            m1 = cst.tile((P, TCH), f32, name="m1", tag="m1")
            nc.vector.reduce_max(m1[:], prb[:], axis=AX.X)
            ge1 = cst.tile((P, TCH, E), f32, name="ge1", tag="ge1")
            nc.vector.tensor_tensor(
                ge1[:], prb[:], m1[:, :, None].to_broadcast((P, TCH, E)),
                Alu.is_ge,
            )
            msk = cst.tile((P, TCH, E), f32, name="msk", tag="msk")
            nc.vector.scalar_tensor_tensor(
                msk[:], ge1[:], -1e9, prb[:], op0=Alu.mult, op1=Alu.add
            )
            m2 = cst.tile((P, TCH), f32, name="m2", tag="m2")
            nc.vector.reduce_max(m2[:], msk[:], axis=AX.X)
            ge2 = cst.tile((P, TCH, E), f32, name="ge2", tag="ge2")
            nc.vector.tensor_tensor(
                ge2[:], prb[:], m2[:, :, None].to_broadcast((P, TCH, E)),
                Alu.is_ge,
            )
            wg = cst.tile((P, TCH, E), f32, name="wg", tag="wg")
            nc.vector.tensor_tensor(wg[:], prb[:], ge2[:], Alu.mult)

            # transpose the two local experts' gate columns ([128,1] -> [1,128]
            # each, so every row lands at partition base 0)
            wrow = [
                cst.tile((1, S), bf16, name=f"wrow{el}", tag=f"wrow{el}")
                for el in range(2)
            ]
            for t in range(TCH):
                for el in range(2):
                    wt_ps = psp.tile((1, P), f32, name="wt_ps", tag="lg",
                                     bufs=1)
                    nc.tensor.transpose(wt_ps[:], wg[:, t, el:el + 1], idf)
                    nc.vector.tensor_copy(wrow[el][:, ts(t, P)], wt_ps[:])

            # broadcast local-expert gate rows across partitions (rank-1 mm)
            rep16 = []
            for el in range(2):
                rp_ps = psp.tile((P, S), f32, name="rp_ps", tag="mm")
                nc.tensor.matmul(
                    rp_ps[:], lhsT=ones16, rhs=wrow[el][:],
                    start=True, stop=True,
                )
                rp = cst.tile((P, S), bf16, name=f"rep16_{el}", tag=f"rep16_{el}")
                nc.vector.tensor_copy(rp[:], rp_ps[:])
                rep16.append(rp)

            # ---------- experts (dense, gated) + shared expert ----------
            moe_ps = [
                psp.tile((P, S), f32, name=f"moe_ps{m}", tag="moe", bufs=4)
                for m in range(DC)
            ]

            def gelu_gate(src_ps, rows, gate_rep):
                """g = gelu_tanh(u) [* gate]; u = src_ps[:rows]. bf16 out."""
                g = tmp.tile((P, S), bf16, name="g", tag="g16", bufs=6)
                if not sim_gelu:
                    g0 = tmp.tile((P, S), bf16, name="g0", tag="x2", bufs=3)
                    nc.scalar.activation(g0[:rows], src_ps[:rows],
                                         Act.Gelu_apprx_tanh)
                    if gate_rep is not None:
                        nc.vector.tensor_tensor(g[:rows], g0[:rows],
                                                gate_rep[:rows], Alu.mult)
                    else:
                        nc.vector.tensor_copy(g[:rows], g0[:rows])
                    return g
                u16 = tmp.tile((P, S), bf16, name="u16", tag="u16", bufs=3)
                nc.vector.tensor_copy(u16[:rows], src_ps[:rows])
                x2 = tmp.tile((P, S), bf16, name="x2", tag="x2", bufs=3)
                nc.vector.tensor_tensor(x2[:rows], u16[:rows], u16[:rows],
                                        Alu.mult)
                t1 = tmp.tile((P, S), bf16, name="t1", tag="x2", bufs=3)
                nc.vector.tensor_scalar(
                    t1[:rows], x2[:rows], GELU_A, 1.0,
                    op0=Alu.mult, op1=Alu.add,
                )
                mm_ = tmp.tile((P, S), bf16, name="mm_", tag="x2", bufs=3)
                nc.vector.tensor_tensor(mm_[:rows], u16[:rows], t1[:rows],
                                        Alu.mult)
                sg = tmp.tile((P, S), bf16, name="sg", tag="x2", bufs=3)
                nc.scalar.activation(sg[:rows], mm_[:rows], Act.Sigmoid,
                                     scale=GELU_C)
                if gate_rep is not None:
                    nc.vector.tensor_tensor(sg[:rows], sg[:rows],
                                            gate_rep[:rows], Alu.mult)
                nc.vector.tensor_tensor(g[:rows], u16[:rows], sg[:rows],
                                        Alu.mult)
                return g

            for el in range(2):
                g16 = []
                for fc in range(FCH):
                    up_ps = psp.tile((P, S), f32, name="up_ps", tag="mm")
                    for c in range(DC):
                        nc.tensor.matmul(
                            up_ps[:],
                            lhsT=msl("ek", 0)[:, (el * DC + c) * FF + fc * P:
                                              (el * DC + c) * FF + (fc + 1) * P],
                            rhs=h2T16[c][:],
                            start=(c == 0), stop=(c == DC - 1),
                        )
                    g16.append(gelu_gate(up_ps, P, rep16[el]))
                for m in range(DC):
                    for fc in range(FCH):
                        nc.tensor.matmul(
                            moe_ps[m][:],
                            lhsT=msl("ev", 0)[:, (el * FCH + fc) * Dm + m * P:
                                              (el * FCH + fc) * Dm + (m + 1) * P],
                            rhs=g16[fc][:, :],
                            start=(el == 0 and fc == 0), stop=False,
                        )

            su_ps = psp.tile((SHF, S), f32, name="su_ps", tag="mm")
            for c in range(DC):
                nc.tensor.matmul(
                    su_ps[:], lhsT=msl("sk", c, SHF), rhs=h2T16[c][:],
                    start=(c == 0), stop=(c == DC - 1),
                )
            gs16 = gelu_gate(su_ps, SHF, None)
            for m in range(DC):
                nc.tensor.matmul(
                    moe_ps[m][:], lhsT=msl("sv", m, P)[0:SHF, :],
                    rhs=gs16[:SHF, :],
                    start=False, stop=True,
                )

            # ---------- fold residual (core 0) + ReduceScatter ----------
            rs_in = drp.tile((Dm, S), f32, name="rs_in")
            rs_out = drp.tile((DSH, S), f32, name="rs_out")
            for m in range(DC):
                fin = tmp.tile((P, S), f32, name="fin", tag="t32")
                nc.vector.scalar_tensor_tensor(
                    fin[:], x1T32[m][:], sel32, moe_ps[m][:],
                    op0=Alu.mult, op1=Alu.add,
                )
                nc.sync.dma_start(rs_in[ts(m, P), :], fin[:])
            nc.gpsimd.collective_compute(
                "ReduceScatter", Alu.add,
                replica_groups=[list(range(NCORES))],
                ins=[rs_in[:]], outs=[rs_out[:]],
            )
            nc.sync.dma_start(outT_d[:], rs_out[:])

    nc.compile()
    return nc


def _prep_inputs(inputs):
    """Host-side sharding/layout prep. Returns (in_maps, a1, a2, full_mask)."""
    x = np.asarray(inputs["x"], np.float32)            # [1,S,D]
    attn_mask = np.asarray(inputs["attn_mask"])        # [1,S]
    wq = np.asarray(inputs["wq"], np.float32)
    wk = np.asarray(inputs["wk"], np.float32)
    wv = np.asarray(inputs["wv"], np.float32)
    wo = np.asarray(inputs["wo"], np.float32)
    a1 = float(np.asarray(inputs["a1"]).reshape(-1)[0])
    g1 = np.asarray(inputs["g1"], np.float32).reshape(Dm)
    b1 = np.asarray(inputs["b1"], np.float32).reshape(Dm)
    a2 = float(np.asarray(inputs["a2"]).reshape(-1)[0])
    g2 = np.asarray(inputs["g2"], np.float32).reshape(Dm)
    b2 = np.asarray(inputs["b2"], np.float32).reshape(Dm)
    gate_w = np.asarray(inputs["gate_w"], np.float32)  # [D,E]
    gate_b = np.asarray(inputs["gate_b"], np.float32).reshape(E)
    ek = np.asarray(inputs["ek"], np.float32)          # [E,D,FF]
    ev = np.asarray(inputs["ev"], np.float32)          # [E,FF,D]
    sk = np.asarray(inputs["sk"], np.float32)          # [1,D,FF]
    sv = np.asarray(inputs["sv"], np.float32)          # [1,FF,D]

    xT = np.ascontiguousarray(x[0].T)                  # [D,S]
    # chunk-major pack: [128, 4*512]
    xTp = np.concatenate([xT[i * P:(i + 1) * P, :] for i in range(DC)], axis=1)

    # rope tables (transposed layout: [freq, pos])
    pos = np.arange(S, dtype=np.float32)
    half = HD // 2
    inv = 1.0 / (10000.0 ** (np.arange(half, dtype=np.float32) / half))
    ang = pos[:, None] * inv[None, :]                  # [S, half]
    cosT = np.cos(ang).T.astype(np.float32)            # [32,S]
    sinT = np.sin(ang).T.astype(np.float32)
    cd = np.concatenate([cosT, cosT, cosT, cosT], 0)
    cs = np.concatenate([-sinT, sinT, -sinT, sinT], 0)

    # additive attention mask, exactly as the reference builds it
    causal = np.tril(np.ones((S, S), np.float32))
    am = attn_mask.astype(np.float32)[0]               # [S]
    cm = causal * am[None, :]
    cm[np.arange(S), np.arange(S)] = 1.0
    addmask = -(1.0 - cm) * 1e9                        # [S,S]
    offdiag_needed = any(
        np.any(addmask[i * P:(i + 1) * P, : i * P] != 0.0)
        for i in range(1, TCH)
    )
    full_mask = bool(offdiag_needed)
    if full_mask:
        mblocks = [addmask[i * P:(i + 1) * P, :] for i in range(TCH)]
    else:
        mblocks = [addmask[i * P:(i + 1) * P, i * P:(i + 1) * P]
                   for i in range(TCH)]

    (atn_l, atn_c), (mid_l, mid_c), (moe_l, moe_c), (p32_l, p32_c) = \
        _layouts(full_mask)

    def pack(layout, total, blocks, dtype):
        arr = np.zeros((P, total), dtype)
        for name, data in blocks.items():
            off, cols = layout[name]
            data = np.asarray(data, np.float32)
            assert data.shape[1] == cols, (name, data.shape, cols)
            arr[:data.shape[0], off:off + cols] = data.astype(dtype)
        return arr

    def cat(chunks):
        return np.concatenate(chunks, axis=1)

    wo_pk = cat([wo[i * P:(i + 1) * P, :] for i in range(DC)])
    id128 = np.eye(P, dtype=np.float32)

    mid_pack = pack(mid_l, mid_c, {"wo": wo_pk}, BF)

    common32 = {
        "g1": np.stack([g1[i * P:(i + 1) * P] for i in range(DC)], 1),
        "b1": np.stack([b1[i * P:(i + 1) * P] for i in range(DC)], 1),
        "g2": np.stack([g2[i * P:(i + 1) * P] for i in range(DC)], 1),
        "b2": np.stack([b2[i * P:(i + 1) * P] for i in range(DC)], 1),
        "idf": id128,
    }

    in_maps = []
    for c in range(NCORES):
        hsl = slice(c * HD, (c + 1) * HD)
        wqk = np.concatenate([wq[:, hsl] * 0.125, wk[:, hsl]], axis=1)
        wqk_pk = cat([wqk[i * P:(i + 1) * P, :] for i in range(DC)])
        wv_pk = cat([wv[i * P:(i + 1) * P, hsl] for i in range(DC)])
        atn_pack = pack(atn_l, atn_c, {
            "wqk": wqk_pk, "wv": wv_pk, "cd": cd, "cs": cs,
            "mask": cat(mblocks), "idbf": id128,
            "ones": np.ones((P, P), np.float32),
        }, BF)

        perm = [2 * c, 2 * c + 1] + [e for e in range(E)
                                     if e not in (2 * c, 2 * c + 1)]
        gwp = gate_w[:, perm]
        gbp = gate_b[perm]
        p32_pack = pack(p32_l, p32_c, dict(
            common32,
            gw=cat([gwp[i * P:(i + 1) * P, :] for i in range(DC)]),
            gb=np.tile(gbp, (P, TCH)),
            sel=np.full((P, 1), 1.0 if c == 0 else 0.0, np.float32),
        ), np.float32)

        ek_pk = cat([ek[2 * c + e][i * P:(i + 1) * P, :]
                     for e in range(2) for i in range(DC)])
        ev_pk = cat([ev[2 * c + e][i * P:(i + 1) * P, :]
                     for e in range(2) for i in range(FCH)])
        sk_pk = cat([sk[0][i * P:(i + 1) * P, c * SHF:(c + 1) * SHF]
                     for i in range(DC)])
        moe_pack = pack(moe_l, moe_c, {
            "ek": ek_pk, "ev": ev_pk, "sk": sk_pk,
            "sv": sv[0][c * SHF:(c + 1) * SHF, :],
        }, BF)

        in_maps.append(dict(
            xT=xTp.astype(np.float32),
            atn16=atn_pack, mid16=mid_pack, moe16=moe_pack, p32=p32_pack,
        ))
    return in_maps, a1, a2, full_mask


def kernel(**inputs):
    from concourse import bass_utils

    sim = bool(os.environ.get("BASSK_SIM"))
    sim_gelu = sim or bool(os.environ.get("BASSK_COMPOSED_GELU"))
    in_maps, a1v, a2v, full_mask = _prep_inputs(inputs)
    key = (a1v, a2v, full_mask, sim_gelu)
    if key not in _PROG_CACHE:
        _PROG_CACHE[key] = _build_program(a1v, a2v, full_mask, sim_gelu)
    nc = _PROG_CACHE[key]

    if sim:
        from concourse.bass_interp import MultiCoreSim

        simu = MultiCoreSim(nc, num_cores=NCORES)
        for c in range(NCORES):
            for k, v in in_maps[c].items():
                simu.cores[c].tensor(k)[:] = v
        simu.simulate(check_with_hw=False)
        shards = [np.array(simu.cores[c].tensor("outT")) for c in range(NCORES)]
    else:
        trace = bool(os.environ.get("BASSK_TRACE"))
        res = bass_utils.run_bass_kernel_spmd(
            nc, in_maps, core_ids=list(range(NCORES)), trace=trace
        )
        LAST_INFO["exec_time_ns"] = res.exec_time_ns
        LAST_INFO["profile_json"] = res.profile_json
        shards = [np.asarray(res.results[c]["outT"]) for c in range(NCORES)]

    outT = np.concatenate(shards, axis=0)              # [D,S]
    return np.ascontiguousarray(outT.T).reshape(1, S, Dm).astype(np.float32)
